# revision 1
# baseline (speedup 1.0000x reference)
"""Causal self-attention Trainium2 kernel (8 NeuronCores).

Problem: B=4, S=2048, D=1024, H=16, HD=64, fp32.
    q/k/v = x @ W{q,k,v}.T + b;  split heads;  causal softmax(q k^T/8) v;
    out = attn @ Wo.T + bo.

Sharding: DP=4 over batch x TP=2 over heads. Core c handles batch c//2 and
heads 8*(c%2)..8*(c%2)+7; it computes a partial output projection over its
8 heads' features. The host sums the two TP partials per batch (bo is fed
as zeros to tp=1 cores so it is added exactly once).

Per-core dataflow (all phases software-pipelined via interleaved emission):
  xT [D,S] (host-transposed, fp32r). q/k are produced feature-major
  (qT/kT [512,S] bf16) by matmul(lhsT=W_tile [d,e], rhs=xT [d,s]); v is
  produced token-major [S, 8, 65] bf16 with a ones column per head so the
  PV matmul accumulates attn^T [64,sq] AND the softmax denominator (row 64)
  in one PSUM tile.
  Attention per (head-pair, 512-query-block): scores are computed
  transposed, [sk=128, sq=512] per head, the two heads of a pair on
  disjoint PE row groups (rows 0-63 / 64-127) so their matmuls overlap in
  the array; one ScalarE exp covers both heads' scores [128,1024] (no
  max-subtraction: scores are O(1) here; fp32 exp never overflows).
  Causal masking zeroes invalid entries of diagonal tiles post-exp with a
  GpSimd affine_select (diagonal tiles are processed first so their longer
  chain hides under off-diagonal iterations).
  Normalization: denominator row -> partition-0 copy -> fast-reciprocal
  (custom DVE ops misread non-zero base partitions, hence the copy) ->
  DRAM-bounce DMA broadcast across 64 partitions -> DVE multiply.
  Out-projection: psO [sq=128, e=512] = sum_hp matmul(lhsT=attnT(fp32r),
  rhs=WoT(fp32r)) + bo via a DMA-broadcast tile; interleaved into the
  attention stream per query-block as its tiles finish.

Matmul dtypes: projections fp32r (2 cyc/row on HW, ~1.5e-4 err);
attention q/k/v/exp in bf16 (frees SBUF for deeper pipelining; final
rel err ~2e-3). PSUM accumulation is always fp32.
"""

import numpy as np

import concourse.bass as bass
import concourse.mybir as mybir
import concourse.tile as tile
from concourse import bacc
from concourse.bass_utils import run_bass_kernel_spmd

B, S, D, H, HD = 4, 2048, 1024, 16, 64
SCALE = HD ** -0.5
N_CORES = 8
HLOC = H // 2          # 8 heads per core
FEAT = HLOC * HD       # 512 features per core
NDT = D // 128         # 8 contraction tiles
NFT = FEAT // 128      # 4 feature tiles
NQB = S // 512         # 4 query blocks of 512
NST = S // 128         # 16 token tiles of 128

F32 = mybir.dt.float32
F32R = mybir.dt.float32r
BF16 = mybir.dt.bfloat16
EXP = mybir.ActivationFunctionType.Exp

_NC_CACHE = {}


def build_nc():
    if "nc" in _NC_CACHE:
        return _NC_CACHE["nc"]
    from contextlib import ExitStack
    from collections import deque
    nc = bacc.Bacc("TRN2", target_bir_lowering=False, debug=False)

    xT = nc.dram_tensor("xT", [D, S], F32R, kind="ExternalInput")
    wqT = nc.dram_tensor("wqT", [D, FEAT], F32R, kind="ExternalInput")
    wkT = nc.dram_tensor("wkT", [D, FEAT], F32R, kind="ExternalInput")
    wvT = nc.dram_tensor("wvT", [D, FEAT], F32R, kind="ExternalInput")
    bqT = nc.dram_tensor("bqT", [FEAT, 1], F32, kind="ExternalInput")
    bkT = nc.dram_tensor("bkT", [FEAT, 1], F32, kind="ExternalInput")
    bv = nc.dram_tensor("bv", [1, FEAT], F32, kind="ExternalInput")
    woT = nc.dram_tensor("woT", [FEAT, D], F32R, kind="ExternalInput")
    bo = nc.dram_tensor("bo", [1, D], F32, kind="ExternalInput")
    out_p = nc.dram_tensor("out_p", [S, D], F32, kind="ExternalOutput")

    with tile.TileContext(nc) as tc:
        with tc.tile_pool(name="ps", bufs=4, space="PSUM") as psp, \
             tc.tile_pool(name="ps2", bufs=2, space="PSUM") as psp2, \
             tc.tile_pool(name="consts", bufs=1) as cpool, \
             tc.tile_pool(name="qk", bufs=1) as qkp, \
             tc.tile_pool(name="vt", bufs=1) as vtp, \
             tc.tile_pool(name="atp", bufs=8) as atp, \
             tc.tile_pool(name="wop", bufs=1) as wop, \
             tc.tile_pool(name="osp", bufs=4) as osp, \
             tc.tile_pool(name="esp", bufs=6) as esp, \
             tc.tile_pool(name="recp", bufs=2) as recp, \
             tc.tile_pool(name="recd", bufs=4, space="DRAM") as recdp, \
             tc.tile_pool(name="bcp", bufs=2) as bcp:

            # ---- constants ----
            vone = cpool.tile([128, HLOC, 1], F32)
            nc.vector.memset(vone, 1.0)
            bvb = cpool.tile([128, FEAT], F32)
            nc.gpsimd.dma_start(out=bvb, in_=bv[:, :].to_broadcast([128, FEAT]))
            bob = cpool.tile([128, D], F32)
            nc.gpsimd.dma_start(out=bob, in_=bo[:, :].to_broadcast([128, D]))
            bq_sb = cpool.tile([128, NFT], F32)
            nc.sync.dma_start(
                out=bq_sb, in_=bqT[:, :].rearrange("(f p) o -> p (f o)", p=128))
            bk_sb = cpool.tile([128, NFT], F32)
            nc.sync.dma_start(
                out=bk_sb, in_=bkT[:, :].rearrange("(f p) o -> p (f o)", p=128))

            # ---- long-lived activation tiles (attention side in bf16) ----
            qt = [[qkp.tile([128, 512], BF16, name=f"qt{f}_{t}")
                   for t in range(NQB)] for f in range(NFT)]
            kt = [[qkp.tile([128, 512], BF16, name=f"kt{f}_{t}")
                   for t in range(NQB)] for f in range(NFT)]
            vt = [vtp.tile([128, HLOC, HD + 1], BF16, name=f"vt{st}")
                  for st in range(NST)]
            at = {}
            wo_sb = []

            def gen_load_wo():
                for hp in range(NFT):
                    woti = wop.tile([128, D], F32R, name=f"wo{hp}")
                    nc.scalar.dma_start(out=woti,
                                        in_=woT[128 * hp:128 * hp + 128, :])
                    wo_sb.append(woti)
                    yield

            # ================= projections (emitted interleaved) ==========
            proj_ctx = ExitStack()
            xtp = proj_ctx.enter_context(tc.tile_pool(name="xtp", bufs=16))
            wqkp = proj_ctx.enter_context(tc.tile_pool(name="wqk", bufs=1))
            wvp = proj_ctx.enter_context(tc.tile_pool(name="wvp", bufs=1))

            wtiles = {}

            def load_w(pname, wsrc, f):
                for d in range(NDT):
                    wti = wqkp.tile([128, 128], F32R, name=f"w{pname}{f}_{d}")
                    nc.scalar.dma_start(
                        out=wti,
                        in_=wsrc[128 * d:128 * d + 128, 128 * f:128 * f + 128])
                    wtiles[pname, f, d] = wti

            wv_sb = []

            def load_wv():
                for d in range(NDT):
                    wvt = wvp.tile([128, FEAT], F32R, name=f"wv{d}")
                    nc.scalar.dma_start(out=wvt,
                                        in_=wvT[128 * d:128 * d + 128, :])
                    wv_sb.append(wvt)

            def gen_proj_t4(t4):
                ts_ = slice(512 * t4, 512 * t4 + 512)
                xts = []
                for d in range(NDT):
                    xti = xtp.tile([128, 512], F32R, name=f"xt{t4}_{d}",
                                   tag="xt")
                    nc.sync.dma_start(out=xti,
                                      in_=xT[128 * d:128 * d + 128, ts_])
                    xts.append(xti)
                for pname, dst, bias, wsrc in (
                        ("q", qt, bq_sb, wqT), ("k", kt, bk_sb, wkT)):
                    for f in range(NFT):
                        if t4 == 0:
                            load_w(pname, wsrc, f)
                        ps = psp.tile([128, 512], F32, tag="ps",
                                      name=f"ps_{pname}{t4}_{f}")
                        for d in range(NDT):
                            nc.tensor.matmul(ps, wtiles[pname, f, d], xts[d],
                                             start=(d == 0),
                                             stop=(d == NDT - 1))
                        nc.vector.tensor_scalar_add(
                            dst[f][t4], ps, bias[:, f:f + 1])
                        yield
                if t4 == 0:
                    load_wv()
                for stl in range(4):
                    st = 4 * t4 + stl
                    ps = psp.tile([128, FEAT], F32, tag="ps", name=f"ps_v{st}")
                    for d in range(NDT):
                        nc.tensor.matmul(
                            ps, xts[d][:, 128 * stl:128 * stl + 128],
                            wv_sb[d], start=(d == 0), stop=(d == NDT - 1))
                    nc.vector.tensor_copy(vt[st][:, :, HD:HD + 1], vone)
                    nc.vector.tensor_add(
                        vt[st][:, :, 0:HD],
                        ps.rearrange("p (h c) -> p h c", c=HD),
                        bvb.rearrange("p (h c) -> p h c", c=HD))
                    yield

            # ================= attention + out-proj =======================
            def emit_group(hp, qb, psA):
                h0, h1 = 2 * hp, 2 * hp + 1
                nj = 4 * (qb + 1)
                # diagonal tiles first: their exp+mask chain latency hides
                # under the off-diagonal iterations that follow.
                js = list(range(4 * qb, nj)) + list(range(0, 4 * qb))
                for i, j in enumerate(js):
                    jt, jc = j // 4, 128 * (j % 4)
                    kslc = kt[hp][jt][:, jc:jc + 128]
                    ps2 = psp2.tile([128, 1024], F32, tag="ps2",
                                    name=f"s{hp}_{qb}_{j}")
                    nc.tensor.matmul(ps2[:, 0:512],
                                     kslc[0:64, :], qt[hp][qb][0:64, :],
                                     start=True, stop=True)
                    nc.tensor.matmul(ps2[:, 512:1024],
                                     kslc[64:128, :], qt[hp][qb][64:128, :],
                                     start=True, stop=True)
                    es2 = esp.tile([128, 1024], BF16, tag="es",
                                   name=f"e{hp}_{qb}_{j}")
                    nc.scalar.activation(es2, ps2, EXP, scale=SCALE)
                    jr = j - 4 * qb
                    if jr >= 0:
                        nc.gpsimd.affine_select(
                            out=es2, in_=es2,
                            compare_op=mybir.AluOpType.is_ge,
                            fill=0.0, base=-128 * jr,
                            pattern=[[0, 2], [1, 512]],
                            channel_multiplier=-1)
                    nc.tensor.matmul(psA[0], vt[j][:, h0, :], es2[:, 0:512],
                                     start=(i == 0), stop=(i == nj - 1))
                    nc.tensor.matmul(psA[1], vt[j][:, h1, :],
                                     es2[:, 512:1024],
                                     start=(i == 0), stop=(i == nj - 1))
                    yield
                at[hp, qb] = atp.tile([128, 512], F32R, tag="at",
                                      name=f"at{hp}_{qb}")
                for idx, h in enumerate((h0, h1)):
                    r0 = 64 * (h % 2)
                    den0 = recp.tile([1, 512], F32, tag="den0",
                                     name=f"dn{h}_{qb}")
                    nc.vector.tensor_copy(den0, psA[idx][HD:HD + 1, :])
                    rec = recp.tile([1, 512], F32, tag="rec",
                                    name=f"rec{h}_{qb}")
                    nc.vector.reciprocal_approx_fast(rec, den0)
                    rd = recdp.tile([1, 512], F32, tag="rd",
                                    name=f"rd{h}_{qb}")
                    nc.scalar.dma_start(out=rd, in_=rec)
                    bcast = bcp.tile([64, 512], F32, tag="bc",
                                     name=f"bc{h}_{qb}")
                    nc.scalar.dma_start(
                        out=bcast, in_=rd[:, :].to_broadcast([64, 512]))
                    nc.vector.tensor_mul(
                        at[hp, qb][r0:r0 + 64, :], psA[idx][0:HD, :], bcast)
                    yield

            def gen_outproj(qb4):
                for r4 in range(4):
                    st = 4 * qb4 + r4
                    for e in range(2):
                        es_ = slice(512 * e, 512 * e + 512)
                        psO = psp.tile([128, 512], F32, tag="ps",
                                       name=f"psO{st}_{e}")
                        for hp in range(NFT):
                            nc.tensor.matmul(
                                psO, at[hp, qb4][:, 128 * r4:128 * r4 + 128],
                                wo_sb[hp][:, es_],
                                start=(hp == 0), stop=(hp == NFT - 1))
                        osb = osp.tile([128, 512], F32, tag="osb",
                                       name=f"o{st}_{e}")
                        nc.vector.tensor_add(osb, psO, bob[:, es_])
                        nc.sync.dma_start(
                            out=out_p[128 * st:128 * st + 128, es_], in_=osb)
                        yield

            fillers = deque()

            def filler_step():
                while fillers:
                    if next(fillers[0], "done") == "done":
                        fillers.popleft()
                        continue
                    return True
                return False

            def drain_fillers():
                while filler_step():
                    pass

            def run_group(hp, qb):
                psA = [psp.tile([HD + 1, 512], F32, tag="ps",
                                name=f"pa{h}_{qb}")
                       for h in (2 * hp, 2 * hp + 1)]
                for _ in emit_group(hp, qb, psA):
                    filler_step()

            for _ in gen_proj_t4(0):
                pass
            for _ in gen_proj_t4(1):
                pass
            fillers.append(gen_load_wo())
            fillers.append(gen_proj_t4(2))
            for hp in range(NFT):
                run_group(hp, 0)
            fillers.append(gen_outproj(0))
            fillers.append(gen_proj_t4(3))
            for hp in range(NFT):
                run_group(hp, 1)
            fillers.append(gen_outproj(1))
            drain_fillers()
            proj_ctx.close()
            for hp in range(NFT):
                run_group(hp, 2)
            fillers.append(gen_outproj(2))
            for hp in range(NFT):
                run_group(hp, 3)
            fillers.append(gen_outproj(3))
            drain_fillers()
    nc.finalize()
    _NC_CACHE["nc"] = nc
    return nc


def make_in_maps(x, Wq, bq, Wk, bk, Wv, bv, Wo, bo):
    import ml_dtypes
    bf = ml_dtypes.bfloat16
    in_maps = []
    for c in range(N_CORES):
        b, tp = c // 2, c % 2
        sl = slice(FEAT * tp, FEAT * (tp + 1))
        in_maps.append({
            "xT": np.ascontiguousarray(x[b].T),
            "wqT": np.ascontiguousarray(Wq[sl].T),
            "wkT": np.ascontiguousarray(Wk[sl].T),
            "wvT": np.ascontiguousarray(Wv[sl].T),
            "bqT": np.ascontiguousarray(bq[sl][:, None]),
            "bkT": np.ascontiguousarray(bk[sl][:, None]),
            "bv": np.ascontiguousarray(bv[sl][None, :]),
            "woT": np.ascontiguousarray(Wo[:, sl].T),
            "bo": (bo[None, :] if tp == 0
                   else np.zeros((1, D), np.float32)),
        })
    return in_maps


def run(inputs, trace=False, trace_cores=None):
    nc = build_nc()
    in_maps = make_in_maps(
        inputs["x"], inputs["Wq"], inputs["bq"], inputs["Wk"], inputs["bk"],
        inputs["Wv"], inputs["bv"], inputs["Wo"], inputs["bo"])
    res = run_bass_kernel_spmd(nc, in_maps, list(range(N_CORES)),
                               trace=trace, trace_cores=trace_cores)
    out = np.empty((B, S, D), np.float32)
    for b in range(B):
        out[b] = res.results[2 * b]["out_p"] + res.results[2 * b + 1]["out_p"]
    return out, res


def kernel(**inputs) -> np.ndarray:
    out, _ = run(inputs, trace=False)
    return out



# revision 10
# speedup vs baseline: 1.1758x; 1.1758x over previous
"""Causal self-attention Trainium2 kernel (8 NeuronCores).

Problem: B=4, S=2048, D=1024, H=16, HD=64, fp32.
    q/k/v = x @ W{q,k,v}.T + b;  split heads;  causal softmax(q k^T/8) v;
    out = attn @ Wo.T + bo.

Sharding: DP=4 over batch x TP=2 over heads. Core c handles batch c//2 and
heads 8*(c%2)..8*(c%2)+7; it computes a partial output projection over its
8 heads' features. The host sums the two TP partials per batch (bo is fed
as zeros to tp=1 cores so it is added exactly once).

All matmuls run in bf16 (1 cyc/row on the PE vs 2 for fp32/fp32r; rel err
~3.4e-3 vs the 2e-2 gate). PSUM accumulation is fp32 throughout.

Per-core dataflow (phases software-pipelined via interleaved emission;
the Tile scheduler uses emission order as priority):
  xT [D,S] (host-transposed bf16). q/k are produced feature-major
  (qT/kT [512,S]) by matmul(lhsT=W_tile [d,e], rhs=xT [d,s]); v is
  produced token-major [S, 8, 65] with a ones column per head so the
  PV matmul accumulates attn^T [64,sq] AND the softmax denominator (row 64)
  in one PSUM tile.
  Attention per (head-pair, 512-query-block): scores are computed
  transposed, [sk=128, sq=512] per head, the two heads of a pair on
  disjoint PE row groups (rows 0-63 / 64-127) so their matmuls overlap in
  the array; one ScalarE exp covers both heads' scores (no max-subtraction:
  scores are O(1) here; fp32 exp never overflows).
  Diagonal key-tiles are trimmed: queries entirely below the tile are
  skipped in the score matmuls, the exp, the mask and the PV matmuls
  (~15% of score work is above the causal boundary at this granularity).
  Causal masking zeroes the remaining triangle with a GpSimd
  affine_select (diagonal tiles run first so their longer chain hides
  under the off-diagonal iterations that follow).
  Normalization: denominator row -> partition-0 copy -> fast-reciprocal
  (custom DVE ops misread non-zero base partitions, hence the copy) ->
  DRAM-bounce DMA broadcast across 64 partitions -> DVE multiply.
  Out-projection: psO [sq=128, e=512] = sum_hp matmul(lhsT=attnT,
  rhs=WoT) + bo via a DMA-broadcast tile; emitted as late filler so its
  PE work lands in the ACT-bound tail of the schedule.
  A dummy exp at emission start pulls the ~2.7us ACT table load off the
  critical path.
"""

import numpy as np

import concourse.bass as bass
import concourse.mybir as mybir
import concourse.tile as tile
from concourse import bacc
from concourse.bass_utils import run_bass_kernel_spmd

B, S, D, H, HD = 4, 2048, 1024, 16, 64
SCALE = HD ** -0.5
N_CORES = 8
HLOC = H // 2          # 8 heads per core
FEAT = HLOC * HD       # 512 features per core
NDT = D // 128         # 8 contraction tiles
NFT = FEAT // 128      # 4 feature tiles
NQB = S // 512         # 4 query blocks of 512
NST = S // 128         # 16 token tiles of 128

F32 = mybir.dt.float32
BF16 = mybir.dt.bfloat16
EXP = mybir.ActivationFunctionType.Exp

_NC_CACHE = {}


def build_nc():
    if "nc" in _NC_CACHE:
        return _NC_CACHE["nc"]
    from contextlib import ExitStack
    from collections import deque
    nc = bacc.Bacc("TRN2", target_bir_lowering=False, debug=False)

    xT = nc.dram_tensor("xT", [D, S], BF16, kind="ExternalInput")
    wqT = nc.dram_tensor("wqT", [D, FEAT], BF16, kind="ExternalInput")
    wkT = nc.dram_tensor("wkT", [D, FEAT], BF16, kind="ExternalInput")
    wvT = nc.dram_tensor("wvT", [D, FEAT], BF16, kind="ExternalInput")
    bqT = nc.dram_tensor("bqT", [FEAT, 1], F32, kind="ExternalInput")
    bkT = nc.dram_tensor("bkT", [FEAT, 1], F32, kind="ExternalInput")
    bv = nc.dram_tensor("bv", [1, FEAT], F32, kind="ExternalInput")
    woT = nc.dram_tensor("woT", [FEAT, D], BF16, kind="ExternalInput")
    bo = nc.dram_tensor("bo", [1, D], F32, kind="ExternalInput")
    out_p = nc.dram_tensor("out_p", [S, D], F32, kind="ExternalOutput")

    with tile.TileContext(nc) as tc:
        with tc.tile_pool(name="ps", bufs=4, space="PSUM") as psp, \
             tc.tile_pool(name="ps2", bufs=2, space="PSUM") as psp2, \
             tc.tile_pool(name="consts", bufs=1) as cpool, \
             tc.tile_pool(name="qk", bufs=1) as qkp, \
             tc.tile_pool(name="vt", bufs=1) as vtp, \
             tc.tile_pool(name="atp", bufs=16) as atp, \
             tc.tile_pool(name="wop", bufs=1) as wop, \
             tc.tile_pool(name="osp", bufs=4) as osp, \
             tc.tile_pool(name="esp", bufs=6) as esp, \
             tc.tile_pool(name="recp", bufs=2) as recp, \
             tc.tile_pool(name="recd", bufs=4, space="DRAM") as recdp, \
             tc.tile_pool(name="bcp", bufs=2) as bcp:

            # ---- constants + ACT exp-table warmup ----
            dmy = cpool.tile([1, 8], F32, name="dmy")
            nc.vector.memset(dmy, 0.0)
            dmy2 = cpool.tile([1, 8], F32, name="dmy2")
            nc.scalar.activation(dmy2, dmy, EXP)
            vone = cpool.tile([128, HLOC, 1], F32)
            nc.vector.memset(vone, 1.0)
            bq_sb = cpool.tile([128, NFT], F32)
            nc.sync.dma_start(
                out=bq_sb, in_=bqT[:, :].rearrange("(f p) o -> p (f o)", p=128))
            bk_sb = cpool.tile([128, NFT], F32)
            nc.sync.dma_start(
                out=bk_sb, in_=bkT[:, :].rearrange("(f p) o -> p (f o)", p=128))
            bvb = cpool.tile([128, FEAT], F32)
            nc.gpsimd.dma_start(out=bvb, in_=bv[:, :].to_broadcast([128, FEAT]))
            bob = cpool.tile([128, D], F32)
            nc.gpsimd.dma_start(out=bob, in_=bo[:, :].to_broadcast([128, D]))

            # ---- long-lived activation tiles ----
            qt = [[qkp.tile([128, 512], BF16, name=f"qt{f}_{t}")
                   for t in range(NQB)] for f in range(NFT)]
            kt = [[qkp.tile([128, 512], BF16, name=f"kt{f}_{t}")
                   for t in range(NQB)] for f in range(NFT)]
            vt = [vtp.tile([128, HLOC, HD + 1], BF16, name=f"vt{st}")
                  for st in range(NST)]
            at = {}
            wo_sb = []

            def gen_load_wo():
                woti = wop.tile([128, NFT, D], BF16, name="wo")
                nc.gpsimd.dma_start(
                    out=woti,
                    in_=woT[:, :].rearrange("(h p) e -> p h e", p=128))
                wo_sb.append(woti)
                yield

            # ================= projections (emitted interleaved) ==========
            # Weights/x are loaded as a few large strided DMAs (split so the
            # first q-projection matmuls gate on ~256KB, not megabytes), and
            # all issue on the gpsimd/sync queues: the scalar queue is kept
            # free for exp, the ACT engine being near-saturated.
            proj_ctx = ExitStack()
            xtp = proj_ctx.enter_context(tc.tile_pool(name="xtp", bufs=3))
            wqkp = proj_ctx.enter_context(tc.tile_pool(name="wqk", bufs=1))
            wvp = proj_ctx.enter_context(tc.tile_pool(name="wvp", bufs=1))

            wsb = {}

            def load_w(pname, wsrc, pool, fa, fb, da=0, db=NDT):
                if pname not in wsb:
                    wsb[pname] = pool.tile([128, NDT, FEAT], BF16,
                                           name=f"w{pname}")
                es_ = slice(128 * fa, 128 * fb)
                nc.gpsimd.dma_start(
                    out=wsb[pname][:, da:db, es_],
                    in_=wsrc[128 * da:128 * db, es_].rearrange(
                        "(dt p) e -> p dt e", p=128))

            def wtile(pname, f, d):
                return wsb[pname][:, d, 128 * f:128 * f + 128]

            def load_xt(t4, split=False):
                ts_ = slice(512 * t4, 512 * t4 + 512)
                xta = xtp.tile([128, NDT, 512], BF16, name=f"xt{t4}",
                               tag="xt")
                if split:
                    for dh in range(2):
                        nc.sync.dma_start(
                            out=xta[:, 4 * dh:4 * dh + 4, :],
                            in_=xT[512 * dh:512 * dh + 512, ts_].rearrange(
                                "(dt p) t -> p dt t", p=128))
                else:
                    nc.sync.dma_start(
                        out=xta,
                        in_=xT[:, ts_].rearrange("(dt p) t -> p dt t", p=128))
                return xta

            def gen_qk(t4, fs, xta):
                for f in fs:
                    for pname, dst, bias in (
                            ("q", qt, bq_sb), ("k", kt, bk_sb)):
                        ps = psp.tile([128, 512], F32, tag="ps",
                                      name=f"ps_{pname}{t4}_{f}")
                        for d in range(NDT):
                            nc.tensor.matmul(ps, wtile(pname, f, d),
                                             xta[:, d, :],
                                             start=(d == 0),
                                             stop=(d == NDT - 1))
                        nc.vector.tensor_scalar_add(
                            dst[f][t4], ps, bias[:, f:f + 1])
                        yield

            def gen_v(t4, xta):
                for stl in range(4):
                    st = 4 * t4 + stl
                    ps = psp.tile([128, FEAT], F32, tag="ps", name=f"ps_v{st}")
                    for d in range(NDT):
                        nc.tensor.matmul(
                            ps, xta[:, d, 128 * stl:128 * stl + 128],
                            wsb["v"][:, d, :],
                            start=(d == 0), stop=(d == NDT - 1))
                    nc.vector.tensor_copy(vt[st][:, :, HD:HD + 1], vone)
                    nc.vector.tensor_add(
                        vt[st][:, :, 0:HD],
                        ps.rearrange("p (h c) -> p h c", c=HD),
                        bvb.rearrange("p (h c) -> p h c", c=HD))
                    yield

            def gen_proj_t4(t4):
                xta = load_xt(t4)
                yield from gen_qk(t4, range(NFT), xta)
                yield from gen_v(t4, xta)

            # ================= attention + out-proj =======================
            def emit_group(hp, qb, psA):
                h0, h1 = 2 * hp, 2 * hp + 1
                nj = 4 * (qb + 1)
                # diagonal tiles first: their exp+mask chain latency hides
                # under the off-diagonal iterations that follow.
                js = list(range(4 * qb, nj)) + list(range(0, 4 * qb))
                for i, j in enumerate(js):
                    jt, jc = j // 4, 128 * (j % 4)
                    kslc = kt[hp][jt][:, jc:jc + 128]
                    jr = j - 4 * qb
                    # queries below 128*jr cannot see this key tile; skip
                    # them in scores/exp/mask/PV (es2 cols [0,o) stay junk
                    # but are never consumed).
                    o = 128 * jr if jr > 0 else 0
                    ps2 = psp2.tile([128, 1024], F32, tag="ps2",
                                    name=f"s{hp}_{qb}_{j}")
                    nc.tensor.matmul(ps2[:, o:512],
                                     kslc[0:64, :],
                                     qt[hp][qb][0:64, o:512],
                                     start=True, stop=True)
                    nc.tensor.matmul(ps2[:, 512 + o:1024],
                                     kslc[64:128, :],
                                     qt[hp][qb][64:128, o:512],
                                     start=True, stop=True)
                    es2 = esp.tile([128, 1024], BF16, tag="es",
                                   name=f"e{hp}_{qb}_{j}")
                    if o:
                        pv = ps2.rearrange("p (h q) -> p h q", q=512)[:, :, o:]
                        ev = es2.rearrange("p (h q) -> p h q", q=512)[:, :, o:]
                    else:
                        pv, ev = ps2, es2
                    nc.scalar.activation(ev, pv, EXP, scale=SCALE)
                    if jr >= 0:
                        nc.gpsimd.affine_select(
                            out=ev, in_=ev,
                            compare_op=mybir.AluOpType.is_ge,
                            fill=0.0, base=0,
                            pattern=[[0, 2], [1, 512 - o]],
                            channel_multiplier=-1)
                    nc.tensor.matmul(psA[0][:, o:], vt[j][:, h0, :],
                                     es2[:, o:512],
                                     start=(i == 0), stop=(i == nj - 1))
                    nc.tensor.matmul(psA[1][:, o:], vt[j][:, h1, :],
                                     es2[:, 512 + o:1024],
                                     start=(i == 0), stop=(i == nj - 1))
                    yield
                at[hp, qb] = atp.tile([128, 512], BF16, tag="at",
                                      name=f"at{hp}_{qb}")
                den0 = recp.tile([1, 1024], F32, tag="den0",
                                 name=f"dn{hp}_{qb}")
                nc.vector.tensor_copy(den0[:, 0:512], psA[0][HD:HD + 1, :])
                nc.vector.tensor_copy(den0[:, 512:1024],
                                      psA[1][HD:HD + 1, :])
                rec = recp.tile([1, 1024], F32, tag="rec",
                                name=f"rec{hp}_{qb}")
                nc.vector.reciprocal_approx_fast(rec, den0)
                rd = recdp.tile([1, 1024], F32, tag="rd", name=f"rd{hp}_{qb}")
                nc.sync.dma_start(out=rd, in_=rec)
                bcast = bcp.tile([64, 1024], F32, tag="bc",
                                 name=f"bc{hp}_{qb}")
                nc.gpsimd.dma_start(
                    out=bcast, in_=rd[:, :].to_broadcast([64, 1024]))
                yield
                for idx in range(2):
                    r0 = 64 * idx
                    nc.vector.tensor_mul(
                        at[hp, qb][r0:r0 + 64, :], psA[idx][0:HD, :],
                        bcast[:, 512 * idx:512 * idx + 512])
                    yield

            def gen_outproj(qb4):
                for r4 in range(4):
                    st = 4 * qb4 + r4
                    osb = osp.tile([128, D], F32, tag="osb", name=f"o{st}")
                    for e in range(2):
                        es_ = slice(512 * e, 512 * e + 512)
                        psO = psp.tile([128, 512], F32, tag="ps",
                                       name=f"psO{st}_{e}")
                        for hp in range(NFT):
                            nc.tensor.matmul(
                                psO, at[hp, qb4][:, 128 * r4:128 * r4 + 128],
                                wo_sb[0][:, hp, es_],
                                start=(hp == 0), stop=(hp == NFT - 1))
                        nc.vector.tensor_add(osb[:, es_], psO, bob[:, es_])
                        yield
                    nc.sync.dma_start(
                        out=out_p[128 * st:128 * st + 128, :], in_=osb)

            fillers = deque()

            def filler_step():
                while fillers:
                    if next(fillers[0], "done") == "done":
                        fillers.popleft()
                        continue
                    return True
                return False

            def drain_fillers():
                while filler_step():
                    pass

            def run_group(hp, qb):
                psA = [psp.tile([HD + 1, 512], F32, tag="ps",
                                name=f"pa{h}_{qb}")
                       for h in (2 * hp, 2 * hp + 1)]
                for _ in emit_group(hp, qb, psA):
                    filler_step()

            # critical startup: just enough projections for group (0, 0)
            xta0 = load_xt(0, split=True)
            load_w("q", wqT, wqkp, 0, 1)
            load_w("k", wkT, wqkp, 0, 1)
            load_w("v", wvT, wvp, 0, NFT, 0, 4)
            load_w("v", wvT, wvp, 0, NFT, 4, 8)
            for _ in gen_qk(0, [0], xta0):
                pass
            load_w("q", wqT, wqkp, 1, NFT)
            load_w("k", wkT, wqkp, 1, NFT)
            for _ in gen_v(0, xta0):
                pass
            fillers.append(gen_qk(0, [1, 2, 3], xta0))
            fillers.append(gen_load_wo())
            fillers.append(gen_proj_t4(1))
            for hp in range(NFT):
                run_group(hp, 0)
            drain_fillers()
            fillers.append(gen_proj_t4(2))
            for hp in range(NFT):
                run_group(hp, 1)
            drain_fillers()
            fillers.append(gen_proj_t4(3))
            fillers.append(gen_outproj(0))
            for hp in range(NFT):
                run_group(hp, 2)
            drain_fillers()
            proj_ctx.close()
            fillers.append(gen_outproj(1))
            fillers.append(gen_outproj(2))
            for hp in range(NFT):
                run_group(hp, 3)
            fillers.append(gen_outproj(3))
            drain_fillers()
    nc.finalize()
    _NC_CACHE["nc"] = nc
    return nc


def make_in_maps(x, Wq, bq, Wk, bk, Wv, bv, Wo, bo):
    import ml_dtypes
    bf = ml_dtypes.bfloat16
    in_maps = []
    for c in range(N_CORES):
        b, tp = c // 2, c % 2
        sl = slice(FEAT * tp, FEAT * (tp + 1))
        in_maps.append({
            "xT": np.ascontiguousarray(x[b].T.astype(bf)),
            "wqT": np.ascontiguousarray(Wq[sl].T.astype(bf)),
            "wkT": np.ascontiguousarray(Wk[sl].T.astype(bf)),
            "wvT": np.ascontiguousarray(Wv[sl].T.astype(bf)),
            "bqT": np.ascontiguousarray(bq[sl][:, None]),
            "bkT": np.ascontiguousarray(bk[sl][:, None]),
            "bv": np.ascontiguousarray(bv[sl][None, :]),
            "woT": np.ascontiguousarray(Wo[:, sl].T.astype(bf)),
            "bo": (bo[None, :] if tp == 0
                   else np.zeros((1, D), np.float32)),
        })
    return in_maps


def run(inputs, trace=False, trace_cores=None):
    nc = build_nc()
    in_maps = make_in_maps(
        inputs["x"], inputs["Wq"], inputs["bq"], inputs["Wk"], inputs["bk"],
        inputs["Wv"], inputs["bv"], inputs["Wo"], inputs["bo"])
    res = run_bass_kernel_spmd(nc, in_maps, list(range(N_CORES)),
                               trace=trace, trace_cores=trace_cores)
    out = np.empty((B, S, D), np.float32)
    for b in range(B):
        out[b] = res.results[2 * b]["out_p"] + res.results[2 * b + 1]["out_p"]
    return out, res


def kernel(**inputs) -> np.ndarray:
    out, _ = run(inputs, trace=False)
    return out


# revision 15
# speedup vs baseline: 1.1971x; 1.0181x over previous
"""Causal self-attention Trainium2 kernel (8 NeuronCores).

Problem: B=4, S=2048, D=1024, H=16, HD=64, fp32.
    q/k/v = x @ W{q,k,v}.T + b;  split heads;  causal softmax(q k^T/8) v;
    out = attn @ Wo.T + bo.

Sharding: DP=4 over batch x TP=2 over heads. Core c handles batch c//2 and
heads 8*(c%2)..8*(c%2)+7; it computes a partial output projection over its
8 heads' features. The host sums the two TP partials per batch (bo is fed
as zeros to tp=1 cores so it is added exactly once).

All matmuls run in bf16 (1 cyc/row on the PE vs 2 for fp32/fp32r; rel err
~3.4e-3 vs the 2e-2 gate). PSUM accumulation is fp32 throughout.

Per-core dataflow (phases software-pipelined via interleaved emission;
the Tile scheduler uses emission order as priority):
  xT [D,S] (host-transposed bf16). q/k are produced feature-major
  (qT/kT [512,S]) by matmul(lhsT=W_tile [d,e], rhs=xT [d,s]); v is
  produced token-major [S, 8, 65] with a ones column per head so the
  PV matmul accumulates attn^T [64,sq] AND the softmax denominator (row 64)
  in one PSUM tile.
  Attention per (head-pair, 512-query-block): scores are computed
  transposed, [sk=128, sq=512] per head, the two heads of a pair on
  disjoint PE row groups (rows 0-63 / 64-127) so their matmuls overlap in
  the array; one ScalarE exp covers both heads' scores (no max-subtraction:
  scores are O(1) here; fp32 exp never overflows).
  Diagonal key-tiles are trimmed: queries entirely below the tile are
  skipped in the score matmuls, the exp, the mask and the PV matmuls
  (~15% of score work is above the causal boundary at this granularity).
  Causal masking zeroes the remaining triangle with a GpSimd
  affine_select (diagonal tiles run first so their longer chain hides
  under the off-diagonal iterations that follow).
  Normalization: denominator row -> partition-0 copy -> fast-reciprocal
  (custom DVE ops misread non-zero base partitions, hence the copy) ->
  DRAM-bounce DMA broadcast across 64 partitions -> DVE multiply.
  Out-projection: psO [sq=128, e=512] = sum_hp matmul(lhsT=attnT,
  rhs=WoT) + bo via a DMA-broadcast tile; emitted as late filler so its
  PE work lands in the ACT-bound tail of the schedule.
  A dummy exp at emission start pulls the ~2.7us ACT table load off the
  critical path.
"""

import numpy as np

import concourse.bass as bass
import concourse.mybir as mybir
import concourse.tile as tile
from concourse import bacc
from concourse.bass_utils import run_bass_kernel_spmd

B, S, D, H, HD = 4, 2048, 1024, 16, 64
SCALE = HD ** -0.5
N_CORES = 8
HLOC = H // 2          # 8 heads per core
FEAT = HLOC * HD       # 512 features per core
NDT = D // 128         # 8 contraction tiles
NFT = FEAT // 128      # 4 feature tiles
NQB = S // 512         # 4 query blocks of 512
NST = S // 128         # 16 token tiles of 128

F32 = mybir.dt.float32
BF16 = mybir.dt.bfloat16
EXP = mybir.ActivationFunctionType.Exp

_NC_CACHE = {}


def build_nc():
    if "nc" in _NC_CACHE:
        return _NC_CACHE["nc"]
    from contextlib import ExitStack
    from collections import deque
    nc = bacc.Bacc("TRN2", target_bir_lowering=False, debug=False)

    xT = nc.dram_tensor("xT", [D, S], BF16, kind="ExternalInput")
    wqT = nc.dram_tensor("wqT", [D, FEAT], BF16, kind="ExternalInput")
    wkT = nc.dram_tensor("wkT", [D, FEAT], BF16, kind="ExternalInput")
    wvT = nc.dram_tensor("wvT", [D, FEAT], BF16, kind="ExternalInput")
    bqT = nc.dram_tensor("bqT", [FEAT, 1], F32, kind="ExternalInput")
    bkT = nc.dram_tensor("bkT", [FEAT, 1], F32, kind="ExternalInput")
    bv = nc.dram_tensor("bv", [1, FEAT], F32, kind="ExternalInput")
    woT = nc.dram_tensor("woT", [FEAT, D], BF16, kind="ExternalInput")
    bo = nc.dram_tensor("bo", [1, D], F32, kind="ExternalInput")
    out_p = nc.dram_tensor("out_p", [S, D], F32, kind="ExternalOutput")

    with tile.TileContext(nc) as tc:
        with tc.tile_pool(name="ps", bufs=4, space="PSUM") as psp, \
             tc.tile_pool(name="ps2", bufs=2, space="PSUM") as psp2, \
             tc.tile_pool(name="consts", bufs=1) as cpool, \
             tc.tile_pool(name="qk", bufs=1) as qkp, \
             tc.tile_pool(name="vt", bufs=1) as vtp, \
             tc.tile_pool(name="atp", bufs=16) as atp, \
             tc.tile_pool(name="wop", bufs=1) as wop, \
             tc.tile_pool(name="osp", bufs=4) as osp, \
             tc.tile_pool(name="esp", bufs=6) as esp, \
             tc.tile_pool(name="recp", bufs=2) as recp, \
             tc.tile_pool(name="recd", bufs=4, space="DRAM") as recdp, \
             tc.tile_pool(name="bcp", bufs=2) as bcp:

            # ---- constants + ACT exp-table warmup ----
            dmy = cpool.tile([1, 8], F32, name="dmy")
            nc.vector.memset(dmy, 0.0)
            dmy2 = cpool.tile([1, 8], F32, name="dmy2")
            nc.scalar.activation(dmy2, dmy, EXP)
            vone = cpool.tile([128, HLOC, 1], F32)
            nc.vector.memset(vone, 1.0)
            # bias loads/broadcasts ride the scalar hwdge ring: it is idle
            # until the first exp, while sync/gpsimd carry the x/weight
            # loads that gate the first matmuls.
            bq_sb = cpool.tile([128, NFT], F32)
            nc.scalar.dma_start(
                out=bq_sb, in_=bqT[:, :].rearrange("(f p) o -> p (f o)", p=128))
            bk_sb = cpool.tile([128, NFT], F32)
            nc.scalar.dma_start(
                out=bk_sb, in_=bkT[:, :].rearrange("(f p) o -> p (f o)", p=128))
            bvb = cpool.tile([128, FEAT], F32)
            nc.scalar.dma_start(out=bvb,
                                in_=bv[:, :].to_broadcast([128, FEAT]))
            bob = cpool.tile([128, D], F32)
            nc.scalar.dma_start(out=bob, in_=bo[:, :].to_broadcast([128, D]))

            # ---- long-lived activation tiles ----
            qt = [[qkp.tile([128, 512], BF16, name=f"qt{f}_{t}")
                   for t in range(NQB)] for f in range(NFT)]
            kt = [[qkp.tile([128, 512], BF16, name=f"kt{f}_{t}")
                   for t in range(NQB)] for f in range(NFT)]
            vt = [vtp.tile([128, HLOC, HD + 1], BF16, name=f"vt{st}")
                  for st in range(NST)]
            at = {}
            wo_sb = []

            def gen_load_wo():
                woti = wop.tile([128, NFT, D], BF16, name="wo")
                nc.gpsimd.dma_start(
                    out=woti,
                    in_=woT[:, :].rearrange("(h p) e -> p h e", p=128))
                wo_sb.append(woti)
                yield

            # ================= projections (emitted interleaved) ==========
            # Weights/x are loaded as a few large strided DMAs (split so the
            # first q-projection matmuls gate on ~256KB, not megabytes), and
            # all issue on the gpsimd/sync queues: the scalar queue is kept
            # free for exp, the ACT engine being near-saturated.
            proj_ctx = ExitStack()
            xtp = proj_ctx.enter_context(tc.tile_pool(name="xtp", bufs=3))
            wqkp = proj_ctx.enter_context(tc.tile_pool(name="wqk", bufs=1))
            wvp = proj_ctx.enter_context(tc.tile_pool(name="wvp", bufs=1))

            wsb = {}

            def load_w(pname, wsrc, pool, fa, fb, da=0, db=NDT):
                if pname not in wsb:
                    wsb[pname] = pool.tile([128, NDT, FEAT], BF16,
                                           name=f"w{pname}")
                es_ = slice(128 * fa, 128 * fb)
                nc.gpsimd.dma_start(
                    out=wsb[pname][:, da:db, es_],
                    in_=wsrc[128 * da:128 * db, es_].rearrange(
                        "(dt p) e -> p dt e", p=128))

            def wtile(pname, f, d):
                return wsb[pname][:, d, 128 * f:128 * f + 128]

            def load_xt(t4, split=False):
                ts_ = slice(512 * t4, 512 * t4 + 512)
                xta = xtp.tile([128, NDT, 512], BF16, name=f"xt{t4}",
                               tag="xt")
                if split:
                    for dh in range(2):
                        nc.sync.dma_start(
                            out=xta[:, 4 * dh:4 * dh + 4, :],
                            in_=xT[512 * dh:512 * dh + 512, ts_].rearrange(
                                "(dt p) t -> p dt t", p=128))
                else:
                    nc.sync.dma_start(
                        out=xta,
                        in_=xT[:, ts_].rearrange("(dt p) t -> p dt t", p=128))
                return xta

            def gen_qk(t4, fs, xta):
                for f in fs:
                    for pname, dst, bias in (
                            ("q", qt, bq_sb), ("k", kt, bk_sb)):
                        ps = psp.tile([128, 512], F32, tag="ps",
                                      name=f"ps_{pname}{t4}_{f}")
                        for d in range(NDT):
                            nc.tensor.matmul(ps, wtile(pname, f, d),
                                             xta[:, d, :],
                                             start=(d == 0),
                                             stop=(d == NDT - 1))
                        nc.vector.tensor_scalar_add(
                            dst[f][t4], ps, bias[:, f:f + 1])
                        yield

            def gen_v(t4, xta):
                for stl in range(4):
                    st = 4 * t4 + stl
                    ps = psp.tile([128, FEAT], F32, tag="ps", name=f"ps_v{st}")
                    for d in range(NDT):
                        nc.tensor.matmul(
                            ps, xta[:, d, 128 * stl:128 * stl + 128],
                            wsb["v"][:, d, :],
                            start=(d == 0), stop=(d == NDT - 1))
                    nc.vector.tensor_copy(vt[st][:, :, HD:HD + 1], vone)
                    nc.vector.tensor_add(
                        vt[st][:, :, 0:HD],
                        ps.rearrange("p (h c) -> p h c", c=HD),
                        bvb.rearrange("p (h c) -> p h c", c=HD))
                    yield

            def gen_proj_t4(t4):
                xta = load_xt(t4)
                yield from gen_qk(t4, range(NFT), xta)
                yield from gen_v(t4, xta)

            # ================= attention + out-proj =======================
            def emit_group(hp, qb, psA):
                h0, h1 = 2 * hp, 2 * hp + 1
                nj = 4 * (qb + 1)
                # diagonal tiles first: their exp+mask chain latency hides
                # under the off-diagonal iterations that follow.
                js = list(range(4 * qb, nj)) + list(range(0, 4 * qb))
                for i, j in enumerate(js):
                    jt, jc = j // 4, 128 * (j % 4)
                    kslc = kt[hp][jt][:, jc:jc + 128]
                    jr = j - 4 * qb
                    # queries below 128*jr cannot see this key tile; skip
                    # them in scores/exp/mask/PV (es2 cols [0,o) stay junk
                    # but are never consumed).
                    o = 128 * jr if jr > 0 else 0
                    ps2 = psp2.tile([128, 1024], F32, tag="ps2",
                                    name=f"s{hp}_{qb}_{j}")
                    nc.tensor.matmul(ps2[:, o:512],
                                     kslc[0:64, :],
                                     qt[hp][qb][0:64, o:512],
                                     start=True, stop=True)
                    nc.tensor.matmul(ps2[:, 512 + o:1024],
                                     kslc[64:128, :],
                                     qt[hp][qb][64:128, o:512],
                                     start=True, stop=True)
                    es2 = esp.tile([128, 1024], BF16, tag="es",
                                   name=f"e{hp}_{qb}_{j}")
                    if o:
                        pv = ps2.rearrange("p (h q) -> p h q", q=512)[:, :, o:]
                        ev = es2.rearrange("p (h q) -> p h q", q=512)[:, :, o:]
                    else:
                        pv, ev = ps2, es2
                    nc.scalar.activation(ev, pv, EXP, scale=SCALE)
                    if jr >= 0:
                        nc.gpsimd.affine_select(
                            out=ev, in_=ev,
                            compare_op=mybir.AluOpType.is_ge,
                            fill=0.0, base=0,
                            pattern=[[0, 2], [1, 512 - o]],
                            channel_multiplier=-1)
                    nc.tensor.matmul(psA[0][:, o:], vt[j][:, h0, :],
                                     es2[:, o:512],
                                     start=(i == 0), stop=(i == nj - 1))
                    nc.tensor.matmul(psA[1][:, o:], vt[j][:, h1, :],
                                     es2[:, 512 + o:1024],
                                     start=(i == 0), stop=(i == nj - 1))
                    yield
                at[hp, qb] = atp.tile([128, 512], BF16, tag="at",
                                      name=f"at{hp}_{qb}")
                den0 = recp.tile([1, 1024], F32, tag="den0",
                                 name=f"dn{hp}_{qb}")
                nc.vector.tensor_copy(den0[:, 0:512], psA[0][HD:HD + 1, :])
                nc.vector.tensor_copy(den0[:, 512:1024],
                                      psA[1][HD:HD + 1, :])
                rec = recp.tile([1, 1024], F32, tag="rec",
                                name=f"rec{hp}_{qb}")
                nc.vector.reciprocal_approx_fast(rec, den0)
                # SBUF APs need a nonzero partition step, so the broadcast
                # bounces through DRAM.
                rd = recdp.tile([1, 1024], F32, tag="rd", name=f"rd{hp}_{qb}")
                nc.sync.dma_start(out=rd, in_=rec)
                bcast = bcp.tile([64, 1024], F32, tag="bc",
                                 name=f"bc{hp}_{qb}")
                nc.gpsimd.dma_start(
                    out=bcast, in_=rd[:, :].to_broadcast([64, 1024]))
                yield
                for idx in range(2):
                    r0 = 64 * idx
                    nc.vector.tensor_mul(
                        at[hp, qb][r0:r0 + 64, :], psA[idx][0:HD, :],
                        bcast[:, 512 * idx:512 * idx + 512])
                    yield

            osb3 = {}

            def gen_outproj(qb4, hps=range(NFT), stage2=False):
                for r4 in range(4):
                    st = 4 * qb4 + r4
                    if stage2:
                        osb = osb3[r4]
                    else:
                        osb = osp.tile([128, D], F32, tag="osb",
                                       name=f"o{st}{'a' if qb4 == 3 else ''}")
                        if qb4 == 3:
                            osb3[r4] = osb
                    hl = list(hps)
                    for e in range(2):
                        es_ = slice(512 * e, 512 * e + 512)
                        psO = psp.tile([128, 512], F32, tag="ps",
                                       name=f"psO{st}_{e}{stage2}")
                        for i, hp in enumerate(hl):
                            nc.tensor.matmul(
                                psO, at[hp, qb4][:, 128 * r4:128 * r4 + 128],
                                wo_sb[0][:, hp, es_],
                                start=(i == 0), stop=(i == len(hl) - 1))
                        if stage2:
                            nc.vector.tensor_add(osb[:, es_], osb[:, es_],
                                                 psO)
                        else:
                            nc.vector.tensor_add(osb[:, es_], psO, bob[:, es_])
                        yield
                    if stage2 or qb4 != 3:
                        nc.sync.dma_start(
                            out=out_p[128 * st:128 * st + 128, :], in_=osb)

            fillers = deque()

            def filler_step():
                while fillers:
                    if next(fillers[0], "done") == "done":
                        fillers.popleft()
                        continue
                    return True
                return False

            def drain_fillers():
                while filler_step():
                    pass

            def run_group(hp, qb):
                psA = [psp.tile([HD + 1, 512], F32, tag="ps",
                                name=f"pa{h}_{qb}")
                       for h in (2 * hp, 2 * hp + 1)]
                for _ in emit_group(hp, qb, psA):
                    filler_step()

            # critical startup: just enough projections for group (0, 0)
            xta0 = load_xt(0, split=True)
            load_w("q", wqT, wqkp, 0, 1)
            load_w("k", wkT, wqkp, 0, 1)
            load_w("v", wvT, wvp, 0, NFT, 0, 4)
            load_w("v", wvT, wvp, 0, NFT, 4, 8)
            for _ in gen_qk(0, [0], xta0):
                pass
            load_w("q", wqT, wqkp, 1, NFT)
            load_w("k", wkT, wqkp, 1, NFT)
            for _ in gen_v(0, xta0):
                pass
            fillers.append(gen_qk(0, [1, 2, 3], xta0))
            fillers.append(gen_load_wo())
            fillers.append(gen_proj_t4(1))
            for hp in range(NFT):
                run_group(hp, 0)
            drain_fillers()
            fillers.append(gen_proj_t4(2))
            for hp in range(NFT):
                run_group(hp, 1)
            drain_fillers()
            fillers.append(gen_proj_t4(3))
            fillers.append(gen_outproj(0))
            for hp in range(NFT):
                run_group(hp, 2)
            drain_fillers()
            proj_ctx.close()
            fillers.append(gen_outproj(1))
            fillers.append(gen_outproj(2))
            for hp in range(3):
                run_group(hp, 3)
            # stage 1 of the last out-projection (heads pairs 0-2 + bias)
            # overlaps group (3,3); only the hp=3 matmul + combine remain
            # after the final attention group.
            fillers.append(gen_outproj(3, hps=range(3)))
            run_group(3, 3)
            fillers.append(gen_outproj(3, hps=[3], stage2=True))
            drain_fillers()
    nc.finalize()
    _NC_CACHE["nc"] = nc
    return nc


def make_in_maps(x, Wq, bq, Wk, bk, Wv, bv, Wo, bo):
    import ml_dtypes
    bf = ml_dtypes.bfloat16
    in_maps = []
    for c in range(N_CORES):
        b, tp = c // 2, c % 2
        sl = slice(FEAT * tp, FEAT * (tp + 1))
        in_maps.append({
            "xT": np.ascontiguousarray(x[b].T.astype(bf)),
            "wqT": np.ascontiguousarray(Wq[sl].T.astype(bf)),
            "wkT": np.ascontiguousarray(Wk[sl].T.astype(bf)),
            "wvT": np.ascontiguousarray(Wv[sl].T.astype(bf)),
            "bqT": np.ascontiguousarray(bq[sl][:, None]),
            "bkT": np.ascontiguousarray(bk[sl][:, None]),
            "bv": np.ascontiguousarray(bv[sl][None, :]),
            "woT": np.ascontiguousarray(Wo[:, sl].T.astype(bf)),
            "bo": (bo[None, :] if tp == 0
                   else np.zeros((1, D), np.float32)),
        })
    return in_maps


def run(inputs, trace=False, trace_cores=None):
    nc = build_nc()
    in_maps = make_in_maps(
        inputs["x"], inputs["Wq"], inputs["bq"], inputs["Wk"], inputs["bk"],
        inputs["Wv"], inputs["bv"], inputs["Wo"], inputs["bo"])
    res = run_bass_kernel_spmd(nc, in_maps, list(range(N_CORES)),
                               trace=trace, trace_cores=trace_cores)
    out = np.empty((B, S, D), np.float32)
    for b in range(B):
        out[b] = res.results[2 * b]["out_p"] + res.results[2 * b + 1]["out_p"]
    return out, res


def kernel(**inputs) -> np.ndarray:
    out, _ = run(inputs, trace=False)
    return out
